# revision 6
# baseline (speedup 1.0000x reference)
"""Sparse (chunked-causal | bidirectional-block) GQA attention on 8 trn2 cores.

Full inputs in, full output out. Sharding: core j handles batch b = j // 4 and
kv-heads {2*(j%4), 2*(j%4)+1} (= query heads 4*(j%4) .. 4*(j%4)+3).

The host does all layout work so the device kernel is pure attention math on
DMA-friendly layouts:
  - q/k cast to fp16 (q pre-scaled by 1/sqrt(D)) and pre-transposed to
    [d, s] so QK^T needs no on-device transposes; v cast to fp16 with a ones
    column appended (softmax denominators fall out of the PV matmul).
  - all device inputs/outputs are laid out so every DMA descriptor is >=4KB
    contiguous per partition.
  - the block schedule (which 128x128 blocks exist, trimmed to their true
    column extent) is computed from the actual mask as the union over both
    batch elements (SPMD: one program for all 8 cores); mask data stays
    exact per core/batch.

Per-core bass kernel, per (head, group-of-512-q):
  - S^T[kv, q] via PE matmuls (lhsT = K^T slice, rhs = Q^T cols) into a
    3-bank PSUM tile (pieces split at bank boundaries), fp16 in / fp32 out.
  - one ACT exp per group -> E (fp16, SBUF).
  - partial blocks are packed at the tail of the group's columns, so ONE
    DVE multiply with the (host-packed, batch-exact) 0/1 mask handles them.
  - PV: per block, accumulate matmul lhsT=E-slice, rhs=V_aug tile into a
    2-bank PSUM group tile; ones column gives denominators.
  - normalize: one DVE reciprocal + one broadcast multiply into a
    4-head-interleaved out tile; one output DMA per group of 512 q rows.
"""

import math

import numpy as np

import concourse.bass as bass
import concourse.mybir as mybir
import concourse.tile as tile
from concourse import bacc
from concourse.bass_utils import run_bass_kernel_spmd

B, S, HQ, HKV, D = 2, 2048, 16, 8, 128
TS = 128                  # block tile size (partitions)
NT = S // TS              # 16 q/kv tiles
GROUP_SUBTILES = 4        # q-subtiles per group (512 q rows)
N_GROUPS = NT // GROUP_SUBTILES
BANK_COLS = 512           # fp32 cols per PSUM bank
ST_COLS = 1536            # st tile cols (3 banks; one group fits in one round)
N_CORES = 8
PAIRS_PER_CORE = 2        # kv heads per core
HEADS_PER_CORE = 4        # query heads per core

F16 = mybir.dt.float16
F32 = mybir.dt.float32


# ---------------------------------------------------------------- host masks

def _segment_ids(m):
    """[B, S] 0/1 -> contiguous-run segment ids (0 = not in a run)."""
    mm = m.astype(np.int64)
    padded = np.pad(mm, ((0, 0), (1, 0)))
    boundary = padded[:, 1:] > padded[:, :-1]
    return mm * np.cumsum(boundary, axis=1)


def _allowed_T(bidirectional_mask, chunk):
    """Per-batch allowed mask, transposed: [B, S(kv), S(q)] bool."""
    seg = _segment_ids(np.asarray(bidirectional_mask))
    r = np.arange(S)
    chunk_ok = (r[:, None] // chunk == r[None, :] // chunk) & (r[:, None] >= r[None, :])
    out = np.zeros((B, S, S), dtype=bool)
    for b in range(B):
        bid = (seg[b][:, None] == seg[b][None, :]) & (seg[b][:, None] > 0)
        out[b] = (chunk_ok | bid).T
    return out


class Schedule:
    """Static (union-over-batch) trimmed block schedule, shared by all cores.

    groups[g] = work dict with fields:
      cols: total packed e-columns for the group
      qk:   [(t, e_off, q_abs, n)]        matmul pieces (bank-split)
      mask: (e_lo, mbuf_off, w) or None   single DVE mask multiply
      pv:   {s_local: [(t, e_off, w, p_lo, full)]}  accumulation lists
    """

    def __init__(self, allowed_T):
        blocks = allowed_T.reshape(B, NT, TS, NT, TS)
        b_any = blocks.any(axis=(2, 4))   # [B, t, s]
        b_all = blocks.all(axis=(2, 4))
        self.u_any = b_any.any(axis=0)    # [t, s]
        self.u_all = b_all.all(axis=0)
        self.partial = self.u_any & ~self.u_all
        colmask = blocks.any(axis=(0, 2))  # [t, s, q_in_tile]
        qlo = np.zeros((NT, NT), np.int64)
        qhi = np.zeros((NT, NT), np.int64)
        for t in range(NT):
            for s in range(NT):
                if not self.u_any[t, s]:
                    continue
                c = colmask[t, s]
                lo = int(np.argmax(c))
                hi = TS - int(np.argmax(c[::-1]))
                # snap to a PE-tile-aligned window (out base partition of the
                # PV matmul must be 0/32/64/96 for <=32 rows, 0/64 for <=64)
                if (lo // 32) * 32 + 32 >= hi:
                    lo = (lo // 32) * 32
                    hi = lo + 32
                elif (lo // 64) * 64 + 64 >= hi:
                    lo = (lo // 64) * 64
                    hi = lo + 64
                else:
                    lo, hi = 0, TS
                qlo[t, s] = lo
                qhi[t, s] = hi
        self.qlo, self.qhi = qlo, qhi

        self.mask_slices = []   # ordered (t, abs_lo, abs_hi) -> host buffer cols
        mbuf_off = 0
        self.groups = []
        for g in range(N_GROUPS):
            s0 = g * GROUP_SUBTILES
            # per-t merged segments of contiguous same-partiality blocks
            entries = []
            for t in range(NT):
                blks = [(s, qlo[t, s], qhi[t, s])
                        for s in range(s0, s0 + GROUP_SUBTILES) if self.u_any[t, s]]
                if not blks:
                    continue
                segs = []  # [abs_lo, abs_hi, partial, [(s, abs_lo, abs_hi)]]
                for (s, lo_, hi_) in blks:
                    al, ah = s * TS + lo_, s * TS + hi_
                    p = bool(self.partial[t, s])
                    if segs and segs[-1][2] == p and segs[-1][1] == al:
                        segs[-1][1] = ah
                        segs[-1][3].append((s, al, ah))
                    else:
                        segs.append([al, ah, p, [(s, al, ah)]])
                entries.append((t, segs))

            work = {"cols": 0, "qk": [], "mask": None,
                    "pv": {sl: [] for sl in range(GROUP_SUBTILES)}}
            full_list = [(t, seg) for (t, segs) in entries for seg in segs if not seg[2]]
            part_list = [(t, seg) for (t, segs) in entries for seg in segs if seg[2]]
            off = 0
            for (t, (lo, hi, p, sblks)) in full_list + part_list:
                w = hi - lo
                o, q0, rem = off, lo, w
                while rem > 0:
                    n = min(BANK_COLS - o % BANK_COLS, rem)
                    work["qk"].append((t, o, q0, n))
                    o += n
                    q0 += n
                    rem -= n
                for (s, bl, bh) in sblks:
                    e_off = off + (bl - lo)
                    bw = bh - bl
                    work["pv"][s - s0].append(
                        (t, e_off, bw, bl - s * TS, bw == TS))
                if p:
                    for (s, bl, bh) in sblks:
                        self.mask_slices.append((t, bl, bh))
                off += w
            part_w = sum(seg[1] - seg[0] for (_, seg) in part_list)
            if part_w:
                work["mask"] = (off - part_w, mbuf_off, part_w)
                mbuf_off += part_w
            work["cols"] = off
            assert off <= ST_COLS, f"group {g} cols {off} > {ST_COLS}"
            # full-width blocks first within each subtile list (bank arming)
            for sl in range(GROUP_SUBTILES):
                work["pv"][sl].sort(key=lambda x: (0 if x[4] else 1,))
            self.groups.append(work)

        self.n_mask_cols = mbuf_off

    def mask_data(self, allowed_T_b):
        """[TS, n_mask_cols] fp16 0/1 packed mask buffer for one batch."""
        out = np.zeros((TS, max(self.n_mask_cols, 1)), dtype=np.float16)
        off = 0
        for (t, bl, bh) in self.mask_slices:
            w = bh - bl
            out[:, off:off + w] = allowed_T_b[t * TS:(t + 1) * TS, bl:bh]
            off += w
        return out

    def key(self):
        return (self.u_any.tobytes(), self.u_all.tobytes(),
                self.qlo.tobytes(), self.qhi.tobytes())


# ------------------------------------------------------------- kernel build

def _broadcast_free(ap, n):
    """Append a 0-step free dim of size n to an AP (read-broadcast)."""
    return bass.AP(tensor=ap.tensor, offset=ap.offset, ap=[*ap.ap, [0, n]])


def _split_dim(ap, n0, n1):
    """Split an AP's first free dim of size n0*n1 into (n0, n1)."""
    (pstep, pnum), (fstep, fnum), *rest = ap.ap
    assert fnum == n0 * n1
    return bass.AP(tensor=ap.tensor, offset=ap.offset,
                   ap=[[pstep, pnum], [fstep * n1, n0], [fstep, n1], *rest])


def _build_body(nc, tc, sched: Schedule, tensors, safe_pv=False):
    qT_in, kT_in, v_in, m_in, o_out = tensors
    ctxs = []
    pv_first_mms = []   # (first_inst_name, [other_inst_names]) per PSUM bank

    def pool(*a, **kw):
        p = tc.tile_pool(*a, **kw)
        ctxs.append(p)
        return p.__enter__()

    consts = pool(name="consts", bufs=1)
    ktp = pool(name="ktp", bufs=2)
    qtp = pool(name="qtp", bufs=4)
    vp = pool(name="vp", bufs=1)
    epool = pool(name="epool", bufs=4)
    outp = pool(name="outp", bufs=N_GROUPS)
    small = pool(name="small", bufs=4)
    stp = pool(name="st_psum", bufs=1 if safe_pv else 2, space="PSUM")
    pvp = pool(name="pv_psum", bufs=1, space="PSUM")

    nmask = max(sched.n_mask_cols, 1)
    mask_sb = consts.tile([TS, nmask], F16)

    # loads in half-tiles (1024 cols), ordered so head 0 group 0's operands
    # land first, then masks/v, then the rest in first-use order
    HS = S // 2
    kts = [[None, None] for _ in range(PAIRS_PER_CORE)]
    qts = [[None, None] for _ in range(HEADS_PER_CORE)]

    def load_kt(pair, half):
        t_ = ktp.tile([TS, HS], F16, tag="kt")
        nc.sync.dma_start(out=t_, in_=kT_in[:, pair, half * HS:(half + 1) * HS])
        kts[pair][half] = t_

    def load_qt(head, half):
        t_ = qtp.tile([TS, HS], F16, tag="qt")
        nc.sync.dma_start(out=t_, in_=qT_in[:, head, half * HS:(half + 1) * HS])
        qts[head][half] = t_

    load_kt(0, 0)
    load_qt(0, 0)
    nc.sync.dma_start(out=mask_sb, in_=m_in[:, :])
    v_sb = vp.tile([TS, NT, PAIRS_PER_CORE, D + 1], F16, tag="v")
    nc.sync.dma_start(out=v_sb, in_=v_in[:, :, :, :])
    load_kt(0, 1)
    load_qt(0, 1)
    load_qt(1, 0)
    load_qt(1, 1)
    load_kt(1, 0)
    load_qt(2, 0)
    load_kt(1, 1)
    load_qt(2, 1)
    load_qt(3, 0)
    load_qt(3, 1)

    def kt_slice(pair, t):
        half, tl = divmod(t, NT // 2)
        return kts[pair][half][:, tl * TS:(tl + 1) * TS]

    def qt_slice(head, q0, n):
        half, q = divmod(q0, HS)
        assert q + n <= HS
        return qts[head][half][:, q:q + n]

    out_tiles = [outp.tile([TS, GROUP_SUBTILES, HEADS_PER_CORE, D], F16,
                           name=f"out_{g}", tag="out")
                 for g in range(N_GROUPS)]

    nbank = GROUP_SUBTILES if safe_pv else 2
    per = 1 if safe_pv else 2

    # work items: head-major, group-minor; PV/normalize lag behind QK/exp/mask
    work = []
    for pair in range(PAIRS_PER_CORE):
        for g_head in range(2):
            head = 2 * pair + g_head
            for g in range(N_GROUPS):
                work.append({"head": head, "pair": pair, "g": g,
                             "w": sched.groups[g]})

    def front_mms(w):
        """Thunks for this item's QK matmuls (emitted interleaved with the
        lagged item's PV matmuls so PV weight-loads hide under QK)."""
        gw = w["w"]
        st = stp.tile([TS, ST_COLS], F32, tag="st")
        w["st"] = st
        thunks = []
        for (t, e_off, q0, n) in gw["qk"]:
            def mk(t=t, e_off=e_off, q0=q0, n=n):
                nc.tensor.matmul(
                    st[:, e_off:e_off + n],
                    lhsT=kt_slice(w["pair"], t),
                    rhs=qt_slice(w["head"], q0, n),
                    start=True, stop=True,
                )
            thunks.append(mk)
        return thunks

    def front_tail(w):
        gw = w["w"]
        st = w["st"]
        e = epool.tile([TS, ST_COLS], F16, tag="e")
        nc.scalar.activation(
            e[:, 0:gw["cols"]], st[:, 0:gw["cols"]],
            mybir.ActivationFunctionType.Exp,
        )
        if gw["mask"] is not None:
            (e_lo, moff, mw) = gw["mask"]
            nc.gpsimd.tensor_mul(
                e[:, e_lo:e_lo + mw],
                e[:, e_lo:e_lo + mw],
                mask_sb[:, moff:moff + mw],
            )
        w["e"] = e

    def back_mms(w):
        gw, g, head, pair = w["w"], w["g"], w["head"], w["pair"]
        pv = pvp.tile([TS, nbank, per, BANK_COLS // per], F32,
                      name=f"pv_{head}_{g}", tag="pv")
        w["pv"] = pv
        e = w["e"]
        bank_first = [None] * nbank
        bank_mms = [[] for _ in range(nbank)]
        bank_total = [0] * nbank
        bank_done = [0] * nbank
        for sl in range(GROUP_SUBTILES):
            bank_total[sl // per] += len(gw["pv"][sl])
        thunks = []
        for sl in range(GROUP_SUBTILES):
            bk, sub = divmod(sl, per)
            for (t, e_off, bw, p_lo, full) in gw["pv"][sl]:
                def mk(sl=sl, bk=bk, sub=sub, t=t, e_off=e_off, bw=bw,
                       p_lo=p_lo, full=full):
                    first = bank_first[bk] is None
                    assert not first or full, "bank must be armed by full blk"
                    bank_done[bk] += 1
                    mm = nc.tensor.matmul(
                        pv[p_lo:p_lo + bw, bk, sub, 0:D + 1],
                        lhsT=e[:, e_off:e_off + bw],
                        rhs=v_sb[:, t, pair, 0:D + 1],
                        start=first,
                        stop=bank_done[bk] == bank_total[bk],
                        tile_position=(0, p_lo),
                    )
                    if first:
                        bank_first[bk] = mm.ins.name
                    else:
                        bank_mms[bk].append(mm.ins.name)
                thunks.append(mk)
        w["bank_state"] = (bank_first, bank_mms)
        return thunks

    def back_tail(w):
        g, head = w["g"], w["head"]
        pv = w["pv"]
        (bank_first, bank_mms) = w["bank_state"]
        pv_first_mms.extend(
            (f, o) for f, o in zip(bank_first, bank_mms) if f is not None)
        recip = small.tile([TS, nbank, per], F32, tag="recip")
        nc.vector.reciprocal(recip, pv[:, :, :, D])
        out_t = out_tiles[g]
        out_ap = _split_dim(out_t[:, :, head, :], nbank, per)
        nc.vector.tensor_mul(out_ap, pv[:, :, :, 0:D],
                             _broadcast_free(recip, D))
        if head == HEADS_PER_CORE - 1:
            nc.sync.dma_start(out=o_out[:, g, :, :, :], in_=out_t)

    def interleave(a, b):
        """Merge thunk lists, spreading b (PV) between a (QK) elements."""
        if not b:
            return list(a)
        if not a:
            return list(b)
        out = []
        na, nb = len(a), len(b)
        ia = ib = 0
        while ia < na or ib < nb:
            if ia < na:
                out.append(a[ia])
                ia += 1
            while ib * na <= ia * nb and ib < nb:
                out.append(b[ib])
                ib += 1
        return out

    LAG = min(2, max(1, len(work) - 1))
    n = len(work)
    for i in range(n + LAG):
        fr = front_mms(work[i]) if i < n else []
        bk = back_mms(work[i - LAG]) if i >= LAG else []
        for thunk in interleave(fr, bk):
            thunk()
        if i < n:
            front_tail(work[i])
        if i >= LAG:
            back_tail(work[i - LAG])

    for p in reversed(ctxs):
        p.__exit__(None, None, None)
    return pv_first_mms


def _verify_pv_order(nc, pv_first_mms):
    """Each PSUM bank's start=True matmul must precede its other matmuls in
    the final (scheduled) program order."""
    pos = {}
    i = 0
    for bb in nc.m.functions[0].blocks:
        for ins in bb.instructions:
            pos[ins.name] = i
            i += 1
    for first, others in pv_first_mms:
        p0 = pos.get(first)
        if p0 is None:
            return False
        for o in others:
            po = pos.get(o)
            if po is None or po < p0:
                return False
    return True


def _build_kernel(sched: Schedule, safe_pv: bool = False):
    nc = bacc.Bacc("TRN2", target_bir_lowering=False, debug=False,
                   num_devices=N_CORES, name="sparse_attn")

    qT_in = nc.dram_tensor("qT", [TS, HEADS_PER_CORE, S], F16, kind="ExternalInput")
    kT_in = nc.dram_tensor("kT", [TS, PAIRS_PER_CORE, S], F16, kind="ExternalInput")
    v_in = nc.dram_tensor("vaug", [TS, NT, PAIRS_PER_CORE, D + 1], F16,
                          kind="ExternalInput")
    m_in = nc.dram_tensor("maskb", [TS, max(sched.n_mask_cols, 1)], F16,
                          kind="ExternalInput")
    o_out = nc.dram_tensor("o", [TS, N_GROUPS, GROUP_SUBTILES, HEADS_PER_CORE, D],
                           F16, kind="ExternalOutput")
    tensors = (qT_in, kT_in, v_in, m_in, o_out)

    with tile.TileContext(nc) as tc:
        pv_first_mms = _build_body(nc, tc, sched, tensors, safe_pv=safe_pv)

    nc.compile()
    if not safe_pv and not _verify_pv_order(nc, pv_first_mms):
        return _build_kernel(sched, safe_pv=True)
    return nc


# --------------------------------------------------------------- entry point

_CACHE = {}


def _get_kernel(sched: Schedule):
    key = sched.key()
    if key not in _CACHE:
        _CACHE[key] = _build_kernel(sched)
    return _CACHE[key]


def _shard_inputs(q, k, v, masks_f16):
    scale = 1.0 / math.sqrt(D)
    in_maps = []
    for core in range(N_CORES):
        b = core // 4
        m = core % 4
        qT = np.ascontiguousarray(
            (q[b, :, 4 * m:4 * m + 4, :] * scale).astype(np.float16)
            .transpose(2, 1, 0))                       # [D, 4, S]
        kT = np.ascontiguousarray(
            k[b, :, 2 * m:2 * m + 2, :].astype(np.float16)
            .transpose(2, 1, 0))                       # [D, 2, S]
        vc = v[b, :, 2 * m:2 * m + 2, :].astype(np.float16)
        vaug = np.ones((S, 2, D + 1), dtype=np.float16)
        vaug[:, :, :D] = vc
        vaug = np.ascontiguousarray(
            vaug.reshape(NT, TS, 2, D + 1).transpose(1, 0, 2, 3))  # [TS,NT,2,D+1]
        in_maps.append({
            "qT": qT, "kT": kT, "vaug": vaug, "maskb": masks_f16[b],
        })
    return in_maps


def kernel(q, k, v, bidirectional_mask, chunk_size):
    q = np.asarray(q, dtype=np.float32)
    k = np.asarray(k, dtype=np.float32)
    v = np.asarray(v, dtype=np.float32)
    chunk = int(np.asarray(chunk_size))

    allowed_T = _allowed_T(bidirectional_mask, chunk)
    sched = Schedule(allowed_T)
    nc = _get_kernel(sched)

    masks_f16 = [sched.mask_data(allowed_T[b]) for b in range(B)]
    in_maps = _shard_inputs(q, k, v, masks_f16)

    res = run_bass_kernel_spmd(nc, in_maps, list(range(N_CORES)))

    out = np.empty((B, S, HQ, D), dtype=np.float32)
    for core in range(N_CORES):
        b = core // 4
        m = core % 4
        oc = res.results[core]["o"]      # [TS, N_GROUPS, GROUP_SUBTILES, 4, D]
        oc = oc.transpose(1, 2, 0, 3, 4).reshape(S, HEADS_PER_CORE, D)
        out[b, :, 4 * m:4 * m + 4, :] = oc.astype(np.float32)
    return out


# revision 8
# speedup vs baseline: 1.1644x; 1.1644x over previous
"""Sparse (chunked-causal | bidirectional-block) GQA attention on 8 trn2 cores.

Full inputs in, full output out. Sharding: core j handles batch b = j // 4 and
kv-heads {2*(j%4), 2*(j%4)+1} (= query heads 4*(j%4) .. 4*(j%4)+3).

Split of work:
  - The DEVICE computes attention over the static chunk-causal block
    structure (all 128x128 blocks (t, s) with kv-tile t <= q-tile s in the
    same chunk). Diagonal blocks are masked with batch-exact 0/1 masks
    (causal triangle + any bidirectional-run extras inside the tile);
    off-diagonal in-chunk blocks are always fully allowed.
  - Bidirectional runs that CROSS a 128-row tile boundary create a few
    extra, nearly-empty blocks off that structure. The q columns they touch
    (a handful per batch) are recomputed exactly on the HOST in fp32 and
    overwritten in the output.

The host also does all layout work so the device kernel is pure attention
math on DMA-friendly layouts: q/k cast to fp16 (q pre-scaled by 1/sqrt(D))
and pre-transposed to [d, s]; v cast to fp16 with a ones column appended
(softmax denominators fall out of the PV matmul); every DMA descriptor is
>=4KB contiguous per partition.

Per-core bass kernel, per (head, group-of-512-q) work item:
  - S^T[kv, q] via PE matmuls (lhsT = K^T tile, rhs = Q^T cols) into a
    PSUM tile; full-block pieces packed first, diagonal blocks last, with
    no piece crossing a PSUM bank boundary and every matmul a full
    128-partition tile_position=(0,0) op (uniform PE config, weight loads
    pipeline back-to-back).
  - one ACT exp per item -> E (fp16, SBUF).
  - one DVE multiply applies the packed diagonal masks (contiguous tail).
  - PV: per block, accumulate matmul lhsT=E-slice, rhs=V_aug tile; the
    ones column gives denominators. PV matmuls of the lagged item are
    interleaved between QK matmuls of the current item.
  - normalize: DVE reciprocal + Pool broadcast multiply into a
    4-head-interleaved out tile; one output DMA per group of 512 q rows.
"""

import math

import numpy as np

import concourse.bass as bass
import concourse.mybir as mybir
import concourse.tile as tile
from concourse import bacc
from concourse.bass_utils import run_bass_kernel_spmd

B, S, HQ, HKV, D = 2, 2048, 16, 8, 128
TS = 128                  # block tile size (partitions)
NT = S // TS              # 16 q/kv tiles
GROUP_SUBTILES = 4        # q-subtiles per group (512 q rows)
N_GROUPS = NT // GROUP_SUBTILES
BANK_COLS = 512           # fp32 cols per PSUM bank
ST_COLS = 1536            # st tile cols (3 banks; one group in one round)
N_CORES = 8
PAIRS_PER_CORE = 2        # kv heads per core
HEADS_PER_CORE = 4        # query heads per core

F16 = mybir.dt.float16
F32 = mybir.dt.float32


# ---------------------------------------------------------------- host masks

def _segment_ids(m):
    """[B, S] 0/1 -> contiguous-run segment ids (0 = not in a run)."""
    mm = m.astype(np.int64)
    padded = np.pad(mm, ((0, 0), (1, 0)))
    boundary = padded[:, 1:] > padded[:, :-1]
    return mm * np.cumsum(boundary, axis=1)


def _allowed_T(bidirectional_mask, chunk):
    """Per-batch allowed mask, transposed: [B, S(kv), S(q)] bool."""
    seg = _segment_ids(np.asarray(bidirectional_mask))
    r = np.arange(S)
    chunk_ok = (r[:, None] // chunk == r[None, :] // chunk) & (r[:, None] >= r[None, :])
    out = np.zeros((B, S, S), dtype=bool)
    for b in range(B):
        bid = (seg[b][:, None] == seg[b][None, :]) & (seg[b][:, None] > 0)
        out[b] = (chunk_ok | bid).T
    return out


class Schedule:
    """Device schedule over the static chunk-causal structure; any u_any
    block off that structure is deferred to the host (fix_cols).

    groups[g] = dict with fields:
      cols: total packed e-columns
      qk:   [(t, e_off, q_abs, n)]   matmul pieces, none crossing a bank
      mask: (e_lo, mbuf_off, w)      single DVE mask mult (diag tail)
      pv:   {s_local: [(t, e_off)]}  accumulation lists (all 128-wide)
    """

    def __init__(self, allowed_T, chunk):
        blocks = allowed_T.reshape(B, NT, TS, NT, TS)
        b_any = blocks.any(axis=(2, 4))
        u_any = b_any.any(axis=0)
        tpc = max(chunk // TS, 1)   # tiles per chunk
        tt, ss = np.meshgrid(np.arange(NT), np.arange(NT), indexing="ij")
        causal = (tt // tpc == ss // tpc) & (ss >= tt)

        # host-fix columns: q extents of any allowed block off the structure
        colmask = blocks.any(axis=(0, 2))  # [t, s, q_in_tile]
        fix = np.zeros(S, dtype=bool)
        for t in range(NT):
            for s in range(NT):
                if u_any[t, s] and not causal[t, s]:
                    fix[s * TS:(s + 1) * TS] |= colmask[t, s]
        self.fix_cols = np.nonzero(fix)[0]

        self.mask_slices = []   # ordered t of diag blocks -> host buffer
        mbuf_off = 0
        self.groups = []
        for g in range(N_GROUPS):
            s0 = g * GROUP_SUBTILES
            t_list = [t for t in range(NT)
                      if any(causal[t, s] for s in range(s0, s0 + GROUP_SUBTILES))]
            # full pieces: for t, span of s>t blocks in group; diag last
            fulls = []
            for t in t_list:
                ss_full = [s for s in range(s0, s0 + GROUP_SUBTILES)
                           if causal[t, s] and s != t]
                if ss_full:
                    lo, hi = min(ss_full), max(ss_full) + 1
                    assert ss_full == list(range(lo, hi))
                    fulls.append((t, lo, hi - lo))
            diags = [t for t in t_list if s0 <= t < s0 + GROUP_SUBTILES]

            work = {"cols": 0, "qk": [], "mask": None,
                    "pv": {sl: [] for sl in range(GROUP_SUBTILES)}}
            e_of_block = {}
            off = 0

            def place(t, q_abs, w):
                """Place a w-col piece; split so no part crosses a bank."""
                nonlocal off
                o = off
                rem, q = w, q_abs
                while rem > 0:
                    n = min(BANK_COLS - o % BANK_COLS, rem)
                    work["qk"].append((t, o, q, n))
                    o += n
                    q += n
                    rem -= n
                start = off
                off += w
                return start

            # first-fit-decreasing keeps full pieces bank-aligned
            for (t, lo, nsub) in sorted(fulls, key=lambda x: -x[2]):
                start = place(t, lo * TS, nsub * TS)
                for i in range(nsub):
                    e_of_block[(t, lo + i)] = start + i * TS
            mask_lo = off
            for t in diags:
                e_of_block[(t, t)] = place(t, t * TS, TS)
                self.mask_slices.append(t)
            if off > mask_lo:
                work["mask"] = (mask_lo, mbuf_off, off - mask_lo)
                mbuf_off += off - mask_lo
            work["cols"] = off
            assert off <= ST_COLS, f"group {g}: {off} cols > {ST_COLS}"

            for s in range(s0, s0 + GROUP_SUBTILES):
                for t in range(NT):
                    if causal[t, s]:
                        work["pv"][s - s0].append((t, e_of_block[(t, s)]))
            self.groups.append(work)

        self.n_mask_cols = mbuf_off

    def mask_data(self, allowed_T_b):
        """[TS, n_mask_cols] fp16 0/1 packed diag-mask buffer, one batch."""
        out = np.zeros((TS, max(self.n_mask_cols, 1)), dtype=np.float16)
        for i, t in enumerate(self.mask_slices):
            out[:, i * TS:(i + 1) * TS] = \
                allowed_T_b[t * TS:(t + 1) * TS, t * TS:(t + 1) * TS]
        return out

    def key(self):
        return (tuple(self.mask_slices),
                tuple((g["cols"], tuple(g["qk"])) for g in self.groups))


# ------------------------------------------------------------- kernel build

def _broadcast_free(ap, n):
    """Append a 0-step free dim of size n to an AP (read-broadcast)."""
    return bass.AP(tensor=ap.tensor, offset=ap.offset, ap=[*ap.ap, [0, n]])


def _split_dim(ap, n0, n1):
    """Split an AP's first free dim of size n0*n1 into (n0, n1)."""
    (pstep, pnum), (fstep, fnum), *rest = ap.ap
    assert fnum == n0 * n1
    return bass.AP(tensor=ap.tensor, offset=ap.offset,
                   ap=[[pstep, pnum], [fstep * n1, n0], [fstep, n1], *rest])


def _build_body(nc, tc, sched: Schedule, tensors, safe_pv=False):
    qT_in, kT_in, v_in, m_in, o_out = tensors
    ctxs = []
    pv_first_mms = []

    def pool(*a, **kw):
        p = tc.tile_pool(*a, **kw)
        ctxs.append(p)
        return p.__enter__()

    consts = pool(name="consts", bufs=1)
    ktp = pool(name="ktp", bufs=2 * PAIRS_PER_CORE)
    qtp = pool(name="qtp", bufs=2 * HEADS_PER_CORE)
    vp = pool(name="vp", bufs=1)
    epool = pool(name="epool", bufs=4)
    outp = pool(name="outp", bufs=N_GROUPS)
    small = pool(name="small", bufs=4)
    stp = pool(name="st_psum", bufs=1 if safe_pv else 2, space="PSUM")
    pvp = pool(name="pv_psum", bufs=1, space="PSUM")

    nmask = max(sched.n_mask_cols, 1)
    mask_sb = consts.tile([TS, nmask], F16)

    # loads in half-tiles (1024 cols), first-use order
    HS = S // 2
    kts = [[None, None] for _ in range(PAIRS_PER_CORE)]
    qts = [[None, None] for _ in range(HEADS_PER_CORE)]

    def load_kt(pair, half):
        t_ = ktp.tile([TS, HS], F16, tag="kt")
        nc.sync.dma_start(out=t_, in_=kT_in[:, pair, half * HS:(half + 1) * HS])
        kts[pair][half] = t_

    def load_qt(head, half):
        t_ = qtp.tile([TS, HS], F16, tag="qt")
        nc.sync.dma_start(out=t_, in_=qT_in[:, head, half * HS:(half + 1) * HS])
        qts[head][half] = t_

    load_kt(0, 0)
    load_qt(0, 0)
    nc.sync.dma_start(out=mask_sb, in_=m_in[:, :])
    v_sb = vp.tile([TS, NT, PAIRS_PER_CORE, D + 1], F16, tag="v")
    nc.sync.dma_start(out=v_sb, in_=v_in[:, :, :, :])
    load_kt(0, 1)
    load_qt(0, 1)
    load_qt(1, 0)
    load_qt(1, 1)
    load_kt(1, 0)
    load_qt(2, 0)
    load_kt(1, 1)
    load_qt(2, 1)
    load_qt(3, 0)
    load_qt(3, 1)

    def kt_slice(pair, t):
        half, tl = divmod(t, NT // 2)
        return kts[pair][half][:, tl * TS:(tl + 1) * TS]

    def qt_slice(head, q0, n):
        half, q = divmod(q0, HS)
        assert q + n <= HS
        return qts[head][half][:, q:q + n]

    out_tiles = [outp.tile([TS, GROUP_SUBTILES, HEADS_PER_CORE, D], F16,
                           name=f"out_{g}", tag="out")
                 for g in range(N_GROUPS)]

    nbank = GROUP_SUBTILES if safe_pv else 2
    per = 1 if safe_pv else 2

    work = []
    for pair in range(PAIRS_PER_CORE):
        for g_head in range(2):
            head = 2 * pair + g_head
            for g in range(N_GROUPS):
                work.append({"head": head, "pair": pair, "g": g,
                             "w": sched.groups[g]})

    def front_mms(w):
        gw = w["w"]
        st = stp.tile([TS, ST_COLS], F32, tag="st")
        w["st"] = st
        thunks = []
        for (t, e_off, q0, n) in gw["qk"]:
            def mk(t=t, e_off=e_off, q0=q0, n=n):
                nc.tensor.matmul(
                    st[:, e_off:e_off + n],
                    lhsT=kt_slice(w["pair"], t),
                    rhs=qt_slice(w["head"], q0, n),
                    start=True, stop=True,
                )
            thunks.append(mk)
        return thunks

    def front_tail(w):
        gw = w["w"]
        st = w["st"]
        e = epool.tile([TS, ST_COLS], F16, tag="e")
        nc.scalar.activation(
            e[:, 0:gw["cols"]], st[:, 0:gw["cols"]],
            mybir.ActivationFunctionType.Exp,
        )
        if gw["mask"] is not None:
            (e_lo, moff, mw) = gw["mask"]
            nc.vector.tensor_mul(
                e[:, e_lo:e_lo + mw],
                e[:, e_lo:e_lo + mw],
                mask_sb[:, moff:moff + mw],
            )
        w["e"] = e

    def back_mms(w):
        gw, g, head, pair = w["w"], w["g"], w["head"], w["pair"]
        pv = pvp.tile([TS, nbank, per, BANK_COLS // per], F32,
                      name=f"pv_{head}_{g}", tag="pv")
        w["pv"] = pv
        e = w["e"]
        bank_first = [None] * nbank
        bank_mms = [[] for _ in range(nbank)]
        bank_total = [0] * nbank
        bank_done = [0] * nbank
        for sl in range(GROUP_SUBTILES):
            bank_total[sl // per] += len(gw["pv"][sl])
        thunks = []
        for sl in range(GROUP_SUBTILES):
            bk, sub = divmod(sl, per)
            for (t, e_off) in gw["pv"][sl]:
                def mk(bk=bk, sub=sub, t=t, e_off=e_off):
                    first = bank_first[bk] is None
                    bank_done[bk] += 1
                    mm = nc.tensor.matmul(
                        pv[:, bk, sub, 0:D + 1],
                        lhsT=e[:, e_off:e_off + TS],
                        rhs=v_sb[:, t, pair, 0:D + 1],
                        start=first,
                        stop=bank_done[bk] == bank_total[bk],
                    )
                    if first:
                        bank_first[bk] = mm.ins.name
                    else:
                        bank_mms[bk].append(mm.ins.name)
                thunks.append(mk)
        w["bank_state"] = (bank_first, bank_mms)
        return thunks

    def back_tail(w):
        g, head = w["g"], w["head"]
        pv = w["pv"]
        (bank_first, bank_mms) = w["bank_state"]
        pv_first_mms.extend(
            (f, o) for f, o in zip(bank_first, bank_mms) if f is not None)
        recip = small.tile([TS, nbank, per], F32, tag="recip")
        nc.vector.reciprocal(recip, pv[:, :, :, D])
        out_t = out_tiles[g]
        out_ap = _split_dim(out_t[:, :, head, :], nbank, per)
        nc.vector.tensor_mul(out_ap, pv[:, :, :, 0:D],
                             _broadcast_free(recip, D))
        if head == HEADS_PER_CORE - 1:
            nc.sync.dma_start(out=o_out[:, g, :, :, :], in_=out_t)

    def interleave(a, b):
        if not b:
            return list(a)
        if not a:
            return list(b)
        out = []
        na, nb = len(a), len(b)
        ia = ib = 0
        while ia < na or ib < nb:
            if ia < na:
                out.append(a[ia])
                ia += 1
            while ib * na <= ia * nb and ib < nb:
                out.append(b[ib])
                ib += 1
        return out

    LAG = min(2, max(1, len(work) - 1))
    n = len(work)
    for i in range(n + LAG):
        fr = front_mms(work[i]) if i < n else []
        bk = back_mms(work[i - LAG]) if i >= LAG else []
        for thunk in interleave(fr, bk):
            thunk()
        if i < n:
            front_tail(work[i])
        if i >= LAG:
            back_tail(work[i - LAG])

    for p in reversed(ctxs):
        p.__exit__(None, None, None)
    return pv_first_mms


def _verify_pv_order(nc, pv_first_mms):
    pos = {}
    i = 0
    for bb in nc.m.functions[0].blocks:
        for ins in bb.instructions:
            pos[ins.name] = i
            i += 1
    for first, others in pv_first_mms:
        p0 = pos.get(first)
        if p0 is None:
            return False
        for o in others:
            po = pos.get(o)
            if po is None or po < p0:
                return False
    return True


def _build_kernel(sched: Schedule, safe_pv: bool = False):
    nc = bacc.Bacc("TRN2", target_bir_lowering=False, debug=False,
                   num_devices=N_CORES, name="sparse_attn")

    qT_in = nc.dram_tensor("qT", [TS, HEADS_PER_CORE, S], F16, kind="ExternalInput")
    kT_in = nc.dram_tensor("kT", [TS, PAIRS_PER_CORE, S], F16, kind="ExternalInput")
    v_in = nc.dram_tensor("vaug", [TS, NT, PAIRS_PER_CORE, D + 1], F16,
                          kind="ExternalInput")
    m_in = nc.dram_tensor("maskb", [TS, max(sched.n_mask_cols, 1)], F16,
                          kind="ExternalInput")
    o_out = nc.dram_tensor("o", [TS, N_GROUPS, GROUP_SUBTILES, HEADS_PER_CORE, D],
                           F16, kind="ExternalOutput")
    tensors = (qT_in, kT_in, v_in, m_in, o_out)

    with tile.TileContext(nc) as tc:
        pv_first_mms = _build_body(nc, tc, sched, tensors, safe_pv=safe_pv)

    nc.compile()
    if not safe_pv and not _verify_pv_order(nc, pv_first_mms):
        return _build_kernel(sched, safe_pv=True)
    return nc


# --------------------------------------------------------------- entry point

_CACHE = {}


def _get_kernel(sched: Schedule):
    key = sched.key()
    if key not in _CACHE:
        _CACHE[key] = _build_kernel(sched)
    return _CACHE[key]


def _shard_inputs(q, k, v, masks_f16):
    scale = 1.0 / math.sqrt(D)
    in_maps = []
    for core in range(N_CORES):
        b = core // 4
        m = core % 4
        qT = np.ascontiguousarray(
            (q[b, :, 4 * m:4 * m + 4, :] * scale).astype(np.float16)
            .transpose(2, 1, 0))                       # [D, 4, S]
        kT = np.ascontiguousarray(
            k[b, :, 2 * m:2 * m + 2, :].astype(np.float16)
            .transpose(2, 1, 0))                       # [D, 2, S]
        vc = v[b, :, 2 * m:2 * m + 2, :].astype(np.float16)
        vaug = np.ones((S, 2, D + 1), dtype=np.float16)
        vaug[:, :, :D] = vc
        vaug = np.ascontiguousarray(
            vaug.reshape(NT, TS, 2, D + 1).transpose(1, 0, 2, 3))
        in_maps.append({
            "qT": qT, "kT": kT, "vaug": vaug, "maskb": masks_f16[b],
        })
    return in_maps


def _host_fix(out, q, k, v, allowed_T, cols):
    """Recompute the given q columns exactly (fp32) and overwrite."""
    if len(cols) == 0:
        return
    scale = 1.0 / math.sqrt(D)
    group = HQ // HKV
    for b in range(B):
        qb = q[b, cols, :, :]                          # [R, HQ, D]
        al = allowed_T[b][:, cols].T                   # [R, S(kv)]
        # logits[r, hq, kv]
        kb = np.repeat(k[b], group, axis=1)            # [S, HQ, D]
        logits = np.einsum("rhd,shd->rhs", qb * scale, kb)
        logits = np.where(al[:, None, :], logits, -np.inf)
        mx = logits.max(axis=-1, keepdims=True)
        e = np.exp(logits - mx)
        p = e / e.sum(axis=-1, keepdims=True)
        vb = np.repeat(v[b], group, axis=1)            # [S, HQ, D]
        out[b, cols, :, :] = np.einsum("rhs,shd->rhd", p, vb)


def kernel(q, k, v, bidirectional_mask, chunk_size):
    q = np.asarray(q, dtype=np.float32)
    k = np.asarray(k, dtype=np.float32)
    v = np.asarray(v, dtype=np.float32)
    chunk = int(np.asarray(chunk_size))

    allowed_T = _allowed_T(bidirectional_mask, chunk)
    sched = Schedule(allowed_T, chunk)
    nc = _get_kernel(sched)

    masks_f16 = [sched.mask_data(allowed_T[b]) for b in range(B)]
    in_maps = _shard_inputs(q, k, v, masks_f16)

    res = run_bass_kernel_spmd(nc, in_maps, list(range(N_CORES)))

    out = np.empty((B, S, HQ, D), dtype=np.float32)
    for core in range(N_CORES):
        b = core // 4
        m = core % 4
        oc = res.results[core]["o"]      # [TS, N_GROUPS, GROUP_SUBTILES, 4, D]
        oc = oc.transpose(1, 2, 0, 3, 4).reshape(S, HEADS_PER_CORE, D)
        out[b, :, 4 * m:4 * m + 4, :] = oc.astype(np.float32)

    _host_fix(out, q, k, v, allowed_T, sched.fix_cols)
    return out


# revision 14
# speedup vs baseline: 1.2883x; 1.1064x over previous
"""Sparse (chunked-causal | bidirectional-block) GQA attention on 8 trn2 cores.

Full inputs in, full output out. Sharding: core j handles batch b = j // 4 and
kv-heads {2*(j%4), 2*(j%4)+1} (= query heads 4*(j%4) .. 4*(j%4)+3).

Split of work:
  - The DEVICE computes attention over the static chunk-causal block
    structure (all 128x128 blocks (t, s) with kv-tile t <= q-tile s in the
    same chunk). Diagonal blocks are masked with batch-exact 0/1 masks
    (causal triangle + any bidirectional-run extras inside the tile);
    off-diagonal in-chunk blocks are always fully allowed.
  - Bidirectional runs that CROSS a 128-row tile boundary create a few
    extra, nearly-empty blocks off that structure. The q columns they touch
    (a handful per batch) are recomputed exactly on the HOST in fp32 and
    overwritten in the output.

The host also does all layout work so the device kernel is pure attention
math on DMA-friendly layouts: q/k cast to fp16 (q pre-scaled by 1/sqrt(D))
and pre-transposed to [d, s]; v cast to fp16 with a ones column appended
(softmax denominators fall out of the PV matmul); every DMA descriptor is
>=4KB contiguous per partition.

Per-core bass kernel, per (head, group-of-512-q) work item:
  - S^T[kv, q] via PE matmuls (lhsT = K^T tile, rhs = Q^T cols) into a
    PSUM tile; full-block pieces packed first, diagonal blocks last, with
    no piece crossing a PSUM bank boundary and every matmul a full
    128-partition tile_position=(0,0) op (uniform PE config, weight loads
    pipeline back-to-back).
  - one ACT exp per item -> E (fp16, SBUF).
  - one DVE multiply applies the packed diagonal masks (contiguous tail).
  - PV: per block, accumulate matmul lhsT=E-slice, rhs=V_aug tile; the
    ones column gives denominators. PV matmuls of the lagged item are
    interleaved between QK matmuls of the current item.
  - normalize: DVE reciprocal + Pool broadcast multiply into a
    4-head-interleaved out tile; one output DMA per group of 512 q rows.
"""

import math

import numpy as np

import concourse.bass as bass
import concourse.mybir as mybir
import concourse.tile as tile
from concourse import bacc
from concourse.bass_utils import run_bass_kernel_spmd

B, S, HQ, HKV, D = 2, 2048, 16, 8, 128
TS = 128                  # block tile size (partitions)
NT = S // TS              # 16 q/kv tiles
GROUP_SUBTILES = 4        # q-subtiles per group (512 q rows)
N_GROUPS = NT // GROUP_SUBTILES
BANK_COLS = 512           # fp32 cols per PSUM bank
ST_COLS = 1536            # st tile cols (3 banks; one group in one round)
N_CORES = 8
PAIRS_PER_CORE = 2        # kv heads per core
HEADS_PER_CORE = 4        # query heads per core

F16 = mybir.dt.float16
F32 = mybir.dt.float32


# ---------------------------------------------------------------- host masks

def _segment_ids(m):
    """[B, S] 0/1 -> contiguous-run segment ids (0 = not in a run)."""
    mm = m.astype(np.int64)
    padded = np.pad(mm, ((0, 0), (1, 0)))
    boundary = padded[:, 1:] > padded[:, :-1]
    return mm * np.cumsum(boundary, axis=1)


def _allowed_T(bidirectional_mask, chunk):
    """Per-batch allowed mask, transposed: [B, S(kv), S(q)] bool."""
    seg = _segment_ids(np.asarray(bidirectional_mask))
    r = np.arange(S)
    chunk_ok = (r[:, None] // chunk == r[None, :] // chunk) & (r[:, None] >= r[None, :])
    out = np.zeros((B, S, S), dtype=bool)
    for b in range(B):
        bid = (seg[b][:, None] == seg[b][None, :]) & (seg[b][:, None] > 0)
        out[b] = (chunk_ok | bid).T
    return out


class Schedule:
    """Device schedule over the static chunk-causal structure; any u_any
    block off that structure is deferred to the host (fix_cols).

    groups[g] = dict with fields:
      cols: total packed e-columns
      qk:   [(t, e_off, q_abs, n)]   matmul pieces, none crossing a bank
      mask: (e_lo, mbuf_off, w)      single DVE mask mult (diag tail)
      pv:   {s_local: [(t, e_off)]}  accumulation lists (all 128-wide)
    """

    def __init__(self, allowed_T, chunk):
        blocks = allowed_T.reshape(B, NT, TS, NT, TS)
        b_any = blocks.any(axis=(2, 4))
        u_any = b_any.any(axis=0)
        tpc = max(chunk // TS, 1)   # tiles per chunk
        tt, ss = np.meshgrid(np.arange(NT), np.arange(NT), indexing="ij")
        causal = (tt // tpc == ss // tpc) & (ss >= tt)

        # host-fix columns: q extents of any allowed block off the structure
        colmask = blocks.any(axis=(0, 2))  # [t, s, q_in_tile]
        fix = np.zeros(S, dtype=bool)
        for t in range(NT):
            for s in range(NT):
                if u_any[t, s] and not causal[t, s]:
                    fix[s * TS:(s + 1) * TS] |= colmask[t, s]
        self.fix_cols = np.nonzero(fix)[0]

        self.mask_slices = []   # ordered t of diag blocks -> host buffer
        mbuf_off = 0
        self.groups = []
        for g in range(N_GROUPS):
            s0 = g * GROUP_SUBTILES
            t_list = [t for t in range(NT)
                      if any(causal[t, s] for s in range(s0, s0 + GROUP_SUBTILES))]
            # full pieces: for t, span of s>t blocks in group; diag last
            fulls = []
            for t in t_list:
                ss_full = [s for s in range(s0, s0 + GROUP_SUBTILES)
                           if causal[t, s] and s != t]
                if ss_full:
                    lo, hi = min(ss_full), max(ss_full) + 1
                    assert ss_full == list(range(lo, hi))
                    fulls.append((t, lo, hi - lo))
            diags = [t for t in t_list if s0 <= t < s0 + GROUP_SUBTILES]

            work = {"cols": 0, "qk": [], "mask": None,
                    "pv": {sl: [] for sl in range(GROUP_SUBTILES)}}
            e_of_block = {}
            off = 0

            def place(t, q_abs, w):
                """Place a w-col piece; split so no part crosses a bank."""
                nonlocal off
                o = off
                rem, q = w, q_abs
                while rem > 0:
                    n = min(BANK_COLS - o % BANK_COLS, rem)
                    work["qk"].append((t, o, q, n))
                    o += n
                    q += n
                    rem -= n
                start = off
                off += w
                return start

            # first-fit-decreasing keeps full pieces bank-aligned
            for (t, lo, nsub) in sorted(fulls, key=lambda x: -x[2]):
                start = place(t, lo * TS, nsub * TS)
                for i in range(nsub):
                    e_of_block[(t, lo + i)] = start + i * TS
            mask_lo = off
            for t in diags:
                e_of_block[(t, t)] = place(t, t * TS, TS)
                self.mask_slices.append(t)
            if off > mask_lo:
                work["mask"] = (mask_lo, mbuf_off, off - mask_lo)
                mbuf_off += off - mask_lo
            work["cols"] = off
            assert off <= ST_COLS, f"group {g}: {off} cols > {ST_COLS}"

            for s in range(s0, s0 + GROUP_SUBTILES):
                for t in range(NT):
                    if causal[t, s]:
                        work["pv"][s - s0].append((t, e_of_block[(t, s)]))
            self.groups.append(work)

        self.n_mask_cols = mbuf_off

    def mask_data(self, allowed_T_b):
        """[TS, n_mask_cols] fp16 0/1 packed diag-mask buffer, one batch."""
        out = np.zeros((TS, max(self.n_mask_cols, 1)), dtype=np.float16)
        for i, t in enumerate(self.mask_slices):
            out[:, i * TS:(i + 1) * TS] = \
                allowed_T_b[t * TS:(t + 1) * TS, t * TS:(t + 1) * TS]
        return out

    def key(self):
        return (tuple(self.mask_slices),
                tuple((g["cols"], tuple(g["qk"])) for g in self.groups))


# ------------------------------------------------------------- kernel build

def _broadcast_free(ap, n):
    """Append a 0-step free dim of size n to an AP (read-broadcast)."""
    return bass.AP(tensor=ap.tensor, offset=ap.offset, ap=[*ap.ap, [0, n]])


def _split_dim(ap, n0, n1):
    """Split an AP's first free dim of size n0*n1 into (n0, n1)."""
    (pstep, pnum), (fstep, fnum), *rest = ap.ap
    assert fnum == n0 * n1
    return bass.AP(tensor=ap.tensor, offset=ap.offset,
                   ap=[[pstep, pnum], [fstep * n1, n0], [fstep, n1], *rest])


def _build_body(nc, tc, sched: Schedule, tensors, safe_pv=False):
    qT_in, kT_in, v_in, m_in, o_out = tensors
    ctxs = []
    pv_first_mms = []

    def pool(*a, **kw):
        p = tc.tile_pool(*a, **kw)
        ctxs.append(p)
        return p.__enter__()

    consts = pool(name="consts", bufs=1)
    ktp = pool(name="ktp", bufs=3 * PAIRS_PER_CORE)
    qtp = pool(name="qtp", bufs=3 * HEADS_PER_CORE)
    vp = pool(name="vp", bufs=1)
    epool = pool(name="epool", bufs=4)
    e2pool = pool(name="e2pool", bufs=4)
    outp = pool(name="outp", bufs=N_GROUPS)
    small = pool(name="small", bufs=4)
    stp = pool(name="st_psum", bufs=1 if safe_pv else 2, space="PSUM")
    pvp = pool(name="pv_psum", bufs=1, space="PSUM")

    nmask = max(sched.n_mask_cols, 1)
    mask_sb = consts.tile([TS, nmask], F16)

    # loads in chunks (two 512-col quarters then a 1024-col half per
    # head/pair), first-use order, so the first QK starts as early as the
    # fixed DMA-init latency allows
    QS = S // 4
    CHUNKS = ((0, QS), (QS, QS), (2 * QS, 2 * QS))
    kts = [[None] * 3 for _ in range(PAIRS_PER_CORE)]
    qts = [[None] * 3 for _ in range(HEADS_PER_CORE)]

    def load_kt(pair, ci):
        o, w = CHUNKS[ci]
        t_ = ktp.tile([TS, w], F16, tag=f"kt{ci}")
        nc.sync.dma_start(out=t_, in_=kT_in[:, pair, o:o + w])
        kts[pair][ci] = t_

    def load_qt(head, ci):
        o, w = CHUNKS[ci]
        t_ = qtp.tile([TS, w], F16, tag=f"qt{ci}")
        nc.sync.dma_start(out=t_, in_=qT_in[:, head, o:o + w])
        qts[head][ci] = t_

    load_kt(0, 0)
    load_qt(0, 0)
    nc.sync.dma_start(out=mask_sb, in_=m_in[:, :])
    v_sb = vp.tile([TS, NT, PAIRS_PER_CORE, D + 1], F16, tag="v")
    nc.sync.dma_start(out=v_sb, in_=v_in[:, :, :, :])
    load_kt(0, 1)
    load_qt(0, 1)
    load_kt(0, 2)
    load_qt(0, 2)
    load_qt(1, 0)
    load_qt(1, 1)
    load_qt(1, 2)
    load_kt(1, 0)
    load_qt(2, 0)
    load_kt(1, 1)
    load_qt(2, 1)
    load_kt(1, 2)
    load_qt(2, 2)
    load_qt(3, 0)
    load_qt(3, 1)
    load_qt(3, 2)

    def _chunk_of(q0, n):
        ci = 0 if q0 < QS else (1 if q0 < 2 * QS else 2)
        o, w = CHUNKS[ci]
        assert q0 - o + n <= w
        return ci, q0 - o

    def kt_slice(pair, t):
        ci, o = _chunk_of(t * TS, TS)
        return kts[pair][ci][:, o:o + TS]

    def qt_slice(head, q0, n):
        ci, o = _chunk_of(q0, n)
        return qts[head][ci][:, o:o + n]

    out_tiles = [outp.tile([TS, GROUP_SUBTILES, HEADS_PER_CORE, D], F16,
                           name=f"out_{g}", tag="out")
                 for g in range(N_GROUPS)]

    nbank = GROUP_SUBTILES if safe_pv else 2
    per = 1 if safe_pv else 2

    work = []
    for pair in range(PAIRS_PER_CORE):
        for g_head in range(2):
            head = 2 * pair + g_head
            for g in range(N_GROUPS):
                work.append({"head": head, "pair": pair, "g": g,
                             "w": sched.groups[g]})

    def front_mms(w):
        gw = w["w"]
        st = stp.tile([TS, ST_COLS], F32, tag="st")
        w["st"] = st
        thunks = []
        for (t, e_off, q0, n) in gw["qk"]:
            def mk(t=t, e_off=e_off, q0=q0, n=n):
                nc.tensor.matmul(
                    st[:, e_off:e_off + n],
                    lhsT=kt_slice(w["pair"], t),
                    rhs=qt_slice(w["head"], q0, n),
                    start=True, stop=True,
                )
            thunks.append(mk)
        return thunks

    def front_tail(w):
        gw = w["w"]
        st = w["st"]
        e = epool.tile([TS, ST_COLS], F16, tag="e")
        nc.scalar.activation(
            e[:, 0:gw["cols"]], st[:, 0:gw["cols"]],
            mybir.ActivationFunctionType.Exp,
        )
        w["e"] = e
        w["e2"] = None
        if gw["mask"] is not None:
            # masked diag cols go to a separate tile so full-block PV
            # matmuls depend only on exp, not on the mask multiply
            (e_lo, moff, mw) = gw["mask"]
            e2 = e2pool.tile([TS, BANK_COLS], F16, tag="e2")
            nc.vector.tensor_mul(
                e2[:, 0:mw],
                e[:, e_lo:e_lo + mw],
                mask_sb[:, moff:moff + mw],
            )
            w["e2"] = e2

    def back_mms(w):
        gw, g, head, pair = w["w"], w["g"], w["head"], w["pair"]
        pv = pvp.tile([TS, nbank, per, BANK_COLS // per], F32,
                      name=f"pv_{head}_{g}", tag="pv")
        w["pv"] = pv
        e = w["e"]
        bank_first = [None] * nbank
        bank_mms = [[] for _ in range(nbank)]
        bank_total = [0] * nbank
        bank_done = [0] * nbank
        for sl in range(GROUP_SUBTILES):
            bank_total[sl // per] += len(gw["pv"][sl])
        mask_lo = gw["mask"][0] if gw["mask"] is not None else None
        full_thunks, diag_thunks = [], []
        for sl in range(GROUP_SUBTILES):
            bk, sub = divmod(sl, per)
            for (t, e_off) in gw["pv"][sl]:
                diag = mask_lo is not None and e_off >= mask_lo

                def mk(bk=bk, sub=sub, t=t, e_off=e_off, diag=diag):
                    src = (w["e2"][:, e_off - mask_lo:e_off - mask_lo + TS]
                           if diag else e[:, e_off:e_off + TS])
                    first = bank_first[bk] is None
                    bank_done[bk] += 1
                    mm = nc.tensor.matmul(
                        pv[:, bk, sub, 0:D + 1],
                        lhsT=src,
                        rhs=v_sb[:, t, pair, 0:D + 1],
                        start=first,
                        stop=bank_done[bk] == bank_total[bk],
                    )
                    if first:
                        bank_first[bk] = mm.ins.name
                    else:
                        bank_mms[bk].append(mm.ins.name)
                (diag_thunks if diag else full_thunks).append(mk)
        w["bank_state"] = (bank_first, bank_mms)
        return full_thunks + diag_thunks

    def back_tail(w):
        g, head = w["g"], w["head"]
        pv = w["pv"]
        (bank_first, bank_mms) = w["bank_state"]
        pv_first_mms.extend(
            (f, o) for f, o in zip(bank_first, bank_mms) if f is not None)
        recip = small.tile([TS, nbank, per], F32, tag="recip")
        nc.vector.reciprocal(recip, pv[:, :, :, D])
        out_t = out_tiles[g]
        out_ap = _split_dim(out_t[:, :, head, :], nbank, per)
        nc.vector.tensor_mul(out_ap, pv[:, :, :, 0:D],
                             _broadcast_free(recip, D))
        if head == HEADS_PER_CORE - 1:
            nc.sync.dma_start(out=o_out[:, g, :, :, :], in_=out_t)

    def interleave(a, b):
        if not b:
            return list(a)
        if not a:
            return list(b)
        out = []
        na, nb = len(a), len(b)
        ia = ib = 0
        while ia < na or ib < nb:
            if ia < na:
                out.append(a[ia])
                ia += 1
            while ib * na <= ia * nb and ib < nb:
                out.append(b[ib])
                ib += 1
        return out

    LAG = min(2, max(1, len(work) - 1))
    n = len(work)
    for i in range(n + LAG):
        fr = front_mms(work[i]) if i < n else []
        bk = back_mms(work[i - LAG]) if i >= LAG else []
        for thunk in interleave(fr, bk):
            thunk()
        # back_tail first: its recip/norm must precede the next mask in the
        # in-order DVE queue, else the pv-psum WAR stalls the PE stream
        if i >= LAG:
            back_tail(work[i - LAG])
        if i < n:
            front_tail(work[i])

    for p in reversed(ctxs):
        p.__exit__(None, None, None)
    return pv_first_mms


def _verify_pv_order(nc, pv_first_mms):
    pos = {}
    i = 0
    for bb in nc.m.functions[0].blocks:
        for ins in bb.instructions:
            pos[ins.name] = i
            i += 1
    for first, others in pv_first_mms:
        p0 = pos.get(first)
        if p0 is None:
            return False
        for o in others:
            po = pos.get(o)
            if po is None or po < p0:
                return False
    return True


def _build_kernel(sched: Schedule, safe_pv: bool = False):
    nc = bacc.Bacc("TRN2", target_bir_lowering=False, debug=False,
                   num_devices=N_CORES, name="sparse_attn")

    qT_in = nc.dram_tensor("qT", [TS, HEADS_PER_CORE, S], F16, kind="ExternalInput")
    kT_in = nc.dram_tensor("kT", [TS, PAIRS_PER_CORE, S], F16, kind="ExternalInput")
    v_in = nc.dram_tensor("vaug", [TS, NT, PAIRS_PER_CORE, D + 1], F16,
                          kind="ExternalInput")
    m_in = nc.dram_tensor("maskb", [TS, max(sched.n_mask_cols, 1)], F16,
                          kind="ExternalInput")
    o_out = nc.dram_tensor("o", [TS, N_GROUPS, GROUP_SUBTILES, HEADS_PER_CORE, D],
                           F16, kind="ExternalOutput")
    tensors = (qT_in, kT_in, v_in, m_in, o_out)

    with tile.TileContext(nc) as tc:
        pv_first_mms = _build_body(nc, tc, sched, tensors, safe_pv=safe_pv)

    nc.compile()
    if not safe_pv and not _verify_pv_order(nc, pv_first_mms):
        return _build_kernel(sched, safe_pv=True)
    return nc


# --------------------------------------------------------------- entry point

_CACHE = {}


def _get_kernel(sched: Schedule):
    key = sched.key()
    if key not in _CACHE:
        _CACHE[key] = _build_kernel(sched)
    return _CACHE[key]


def _shard_inputs(q, k, v, masks_f16):
    scale = 1.0 / math.sqrt(D)
    in_maps = []
    for core in range(N_CORES):
        b = core // 4
        m = core % 4
        qT = np.ascontiguousarray(
            (q[b, :, 4 * m:4 * m + 4, :] * scale).astype(np.float16)
            .transpose(2, 1, 0))                       # [D, 4, S]
        kT = np.ascontiguousarray(
            k[b, :, 2 * m:2 * m + 2, :].astype(np.float16)
            .transpose(2, 1, 0))                       # [D, 2, S]
        vc = v[b, :, 2 * m:2 * m + 2, :].astype(np.float16)
        vaug = np.ones((S, 2, D + 1), dtype=np.float16)
        vaug[:, :, :D] = vc
        vaug = np.ascontiguousarray(
            vaug.reshape(NT, TS, 2, D + 1).transpose(1, 0, 2, 3))
        in_maps.append({
            "qT": qT, "kT": kT, "vaug": vaug, "maskb": masks_f16[b],
        })
    return in_maps


def _host_fix(out, q, k, v, allowed_T, cols):
    """Recompute the given q columns exactly (fp32) and overwrite."""
    if len(cols) == 0:
        return
    scale = 1.0 / math.sqrt(D)
    group = HQ // HKV
    for b in range(B):
        qb = q[b, cols, :, :]                          # [R, HQ, D]
        al = allowed_T[b][:, cols].T                   # [R, S(kv)]
        # logits[r, hq, kv]
        kb = np.repeat(k[b], group, axis=1)            # [S, HQ, D]
        logits = np.einsum("rhd,shd->rhs", qb * scale, kb)
        logits = np.where(al[:, None, :], logits, -np.inf)
        mx = logits.max(axis=-1, keepdims=True)
        e = np.exp(logits - mx)
        p = e / e.sum(axis=-1, keepdims=True)
        vb = np.repeat(v[b], group, axis=1)            # [S, HQ, D]
        out[b, cols, :, :] = np.einsum("rhs,shd->rhd", p, vb)


def kernel(q, k, v, bidirectional_mask, chunk_size):
    q = np.asarray(q, dtype=np.float32)
    k = np.asarray(k, dtype=np.float32)
    v = np.asarray(v, dtype=np.float32)
    chunk = int(np.asarray(chunk_size))

    allowed_T = _allowed_T(bidirectional_mask, chunk)
    sched = Schedule(allowed_T, chunk)
    nc = _get_kernel(sched)

    masks_f16 = [sched.mask_data(allowed_T[b]) for b in range(B)]
    in_maps = _shard_inputs(q, k, v, masks_f16)

    res = run_bass_kernel_spmd(nc, in_maps, list(range(N_CORES)))

    out = np.empty((B, S, HQ, D), dtype=np.float32)
    for core in range(N_CORES):
        b = core // 4
        m = core % 4
        oc = res.results[core]["o"]      # [TS, N_GROUPS, GROUP_SUBTILES, 4, D]
        oc = oc.transpose(1, 2, 0, 3, 4).reshape(S, HEADS_PER_CORE, D)
        out[b, :, 4 * m:4 * m + 4, :] = oc.astype(np.float32)

    _host_fix(out, q, k, v, allowed_T, sched.fix_cols)
    return out
